# revision 5
# baseline (speedup 1.0000x reference)
"""Causal multi-head attention (RoPE) forward for Trainium2, sharded over 8 NeuronCores.

Problem (hardcoded): B=2, S=2048, E=128, H=16, D=128, inner=2048.
  out = softmax(causal(rope(q@Wq) @ rope(q@Wk).T / sqrt(D))) @ (q@Wv) @ Wo

Sharding: tensor-parallel over heads — core c owns heads {2c, 2c+1} for both
batches (4 attention units/core). Each core computes its heads' projections,
attention, and partial W_o output (row-shard); host sums the 8 partials.

V2 design (vs baseline): everything fp16, V-stationary AV matmuls (N=512,
accumulated over t-chunks in one PSUM bank) — no PE transposes, no N=129
matmuls. Denominator via DVE fp16 add-tree over exp chunks + one
ones-stationary matmul (broadcast den in PSUM) + DVE reciprocal; normalize
with one DVE mul that also produces the W_o rhs. W_o accumulates both heads
into one PSUM bank. Exp on ACT over [128,1024] PSUM pairs; diagonal chunks
get separate restricted exps and keep persistent-zero invalid regions so the
den tree reads full chunks. Rope: 2 DVE muls (PSUM 1x) + GpSimd add; tril
masks on GpSimd. PSUM: exp 2x[128,1024] + oT/psv 2x[128,512] + den/fin
2x[128,512] = 8 banks. Emission interleaves stage_b(b1) into stage_c(b0)
to keep the PE dense.
"""

import os
import sys
import numpy as np
import ml_dtypes
BF = ml_dtypes.bfloat16

for _p in ("/root/.axon_site", "/root/.axon_site/_ro/trn_rl_repo",
           "/root/.axon_site/_ro/pypackages", "/opt/trn_rl_repo"):
    if os.path.isdir(_p) and _p not in sys.path:
        sys.path.append(_p)

from contextlib import ExitStack

import concourse.bacc as bacc
import concourse.mybir as mybir
import concourse.tile as tile
from concourse import bass_utils

F32 = mybir.dt.float32
F16 = mybir.dt.float16
BF16 = mybir.dt.bfloat16
AF = mybir.ActivationFunctionType

B, S, E = 2, 2048, 128
H, D = 16, 128
NCORES = 8
HPC = H // NCORES          # heads per core = 2
WIN = 512                  # q-window
NW = S // WIN              # windows per batch = 4
SCALE = 1.0 / np.sqrt(D)

_CACHE = {}


def _build():
    nc = bacc.Bacc("TRN2", target_bir_lowering=False, debug=False)

    qT_d = nc.dram_tensor("qT", [E, B * S], F16, kind="ExternalInput").ap()
    wqk_d = nc.dram_tensor("wqk", [E, 8 * D], F16, kind="ExternalInput").ap()
    wv_d = nc.dram_tensor("wv", [E, HPC * D], F16, kind="ExternalInput").ap()
    wo_d = nc.dram_tensor("wo", [D, HPC * E], BF16, kind="ExternalInput").ap()
    cos_d = nc.dram_tensor("cosT", [D, S], F16, kind="ExternalInput").ap()
    sin_d = nc.dram_tensor("sinT", [D, S], F16, kind="ExternalInput").ap()
    tril_d = nc.dram_tensor("tril", [128, 128], BF16, kind="ExternalInput").ap()
    outp_d = nc.dram_tensor("outp", [B * E, S], F32, kind="ExternalOutput").ap()

    with tile.TileContext(nc) as tc, ExitStack() as ctx:
        const = ctx.enter_context(tc.tile_pool(name="const", bufs=1))
        qkp = ctx.enter_context(tc.tile_pool(name="qkp", bufs=1))
        vhp = ctx.enter_context(tc.tile_pool(name="vhp", bufs=1))
        eap = ctx.enter_context(tc.tile_pool(name="eap", bufs=1))
        scrp = ctx.enter_context(tc.tile_pool(name="scrp", bufs=1))
        tmp = ctx.enter_context(tc.tile_pool(name="tmp", bufs=3))
        ps_exp = ctx.enter_context(tc.tile_pool(name="ps_exp", bufs=2, space="PSUM"))
        ps_oT = ctx.enter_context(tc.tile_pool(name="ps_oT", bufs=2, space="PSUM"))
        ps_misc = ctx.enter_context(tc.tile_pool(name="ps_misc", bufs=2, space="PSUM"))

        # ---- constant loads ----
        qt_w = []
        for i in range(B * NW):
            t = const.tile([128, WIN], F16, tag=f"qt{i}", name=f"qt{i}")
            qt_w.append(t)
        wqk_t = const.tile([128, 8 * D], F16, tag="wqk", name="wqk_t")
        wv_t = const.tile([128, HPC * D], F16, tag="wv", name="wv_t")
        wo_t = const.tile([128, HPC * E], BF16, tag="wo", name="wo_t")
        cos_t = const.tile([128, S], F16, tag="cos", name="cos_t")
        sin_t = const.tile([128, S], F16, tag="sin", name="sin_t")
        tril_t = const.tile([128, 128], BF16, tag="tril", name="tril_t")
        ones_t = const.tile([128, 128], BF16, tag="ones", name="ones_t")
        nc.vector.memset(ones_t[:], 1.0)
        # spread DMA triggers over engine queues; critical tensors first
        nc.sync.dma_start(wqk_t[:, 0:512], wqk_d[:, 0:512])
        nc.scalar.dma_start(wqk_t[:, 512:1024], wqk_d[:, 512:1024])
        nc.sync.dma_start(qt_w[0][:, 0:256], qT_d[:, 0:256])
        nc.scalar.dma_start(qt_w[0][:, 256:WIN], qT_d[:, 256:WIN])
        nc.sync.dma_start(qt_w[1][:, 0:256], qT_d[:, WIN:WIN + 256])
        nc.scalar.dma_start(qt_w[1][:, 256:WIN], qT_d[:, WIN + 256:2 * WIN])
        nc.sync.dma_start(cos_t[:, 0:1024], cos_d[:, 0:1024])
        nc.sync.dma_start(sin_t[:, 0:1024], sin_d[:, 0:1024])
        nc.sync.dma_start(wv_t[:], wv_d[:])
        nc.scalar.dma_start(qt_w[4][:], qT_d[:, 4 * WIN:5 * WIN])
        nc.scalar.dma_start(qt_w[5][:], qT_d[:, 5 * WIN:6 * WIN])
        nc.sync.dma_start(cos_t[:, 1024:2048], cos_d[:, 1024:2048])
        nc.sync.dma_start(sin_t[:, 1024:2048], sin_d[:, 1024:2048])
        nc.sync.dma_start(tril_t[:], tril_d[:])
        nc.scalar.dma_start(qt_w[2][:], qT_d[:, 2 * WIN:3 * WIN])
        nc.scalar.dma_start(qt_w[3][:], qT_d[:, 3 * WIN:4 * WIN])
        nc.scalar.dma_start(qt_w[6][:], qT_d[:, 6 * WIN:7 * WIN])
        nc.sync.dma_start(qt_w[7][:], qT_d[:, 7 * WIN:8 * WIN])
        nc.sync.dma_start(wo_t[:], wo_d[:])

        # persistent rope'd projections: (u, kind) -> [128, S] f16 (all windows)
        qk = {}
        for u in range(B * HPC):
            for kind in range(2):
                qk[(u, kind)] = qkp.tile(
                    [128, S], F16, tag=f"qk{u}_{kind}", name=f"qk{u}_{kind}")
        # V per (b, t-chunk): [128, HPC*D] f16 (both heads side by side)
        vh = {}
        for b in range(B):
            for c in range(S // 128):
                vh[(b, c)] = vhp.tile(
                    [128, HPC * D], BF16, tag=f"vh{b}_{c}", name=f"vh{b}_{c}")

        # e_all double-buffered by unit parity: (W, par) -> [128, (4W+4)*WIN]
        # f16. Diag-invalid regions memset to zero ONCE; exp never writes
        # them, so zeros persist across slot reuse (the den tree reads them).
        ea_t = {}
        for W in range(NW):
            nch = 4 * W + 4
            for par in range(2):
                t = eap.tile([128, nch * WIN], BF16, tag=f"ea{W}_{par}",
                             name=f"ea{W}_{par}")
                ea_t[(W, par)] = t
                for s in range(1, 4):
                    tci = 4 * W + s
                    nc.gpsimd.memset(t[:, tci * WIN:tci * WIN + 128 * s], 0.0)
        # den-tree scratch: per (W, par) for big W, shared for small W
        scr_t = {}
        for W in range(NW):
            hw = (4 * W + 4) // 2
            if W >= 2:
                for par in range(2):
                    scr_t[(W, par)] = scrp.tile(
                        [128, hw * WIN], BF16, tag=f"scr{W}_{par}",
                        name=f"scr{W}_{par}")
            else:
                s = scrp.tile([128, hw * WIN], BF16, tag=f"scr{W}",
                              name=f"scr{W}")
                scr_t[(W, 0)] = s
                scr_t[(W, 1)] = s

        fins = {}

        def rope_piece(b, wp, hl, kind, misc=False):
            i0 = b * NW + 2 * wp
            u = b * HPC + hl
            ja = (kind * 4 + hl * 2) * D
            if misc:
                # unpaired [128,512] through ps_misc: keeps ps_exp free for
                # the concurrent stage_c score/exp pipeline
                for dw in range(2):
                    w = 2 * wp + dw
                    slw = slice(w * WIN, (w + 1) * WIN)
                    pa = ps_misc.tile([128, WIN], F32, tag="misc",
                                      name=f"pa{b}_{wp}_{hl}_{kind}_{dw}")
                    nc.tensor.matmul(pa[:], wqk_t[:, ja:ja + D], qt_w[i0 + dw][:])
                    pb = ps_misc.tile([128, WIN], F32, tag="misc",
                                      name=f"pb{b}_{wp}_{hl}_{kind}_{dw}")
                    nc.tensor.matmul(pb[:], wqk_t[:, ja + D:ja + 2 * D],
                                     qt_w[i0 + dw][:])
                    t1 = tmp.tile([128, 2 * WIN], F16, tag="t1",
                                  name=f"t1_{b}_{wp}_{hl}_{kind}_{dw}", bufs=3)
                    nc.vector.tensor_mul(t1[:, 0:WIN], pa[:], cos_t[:, slw])
                    t2 = tmp.tile([128, 2 * WIN], F16, tag="t2",
                                  name=f"t2_{b}_{wp}_{hl}_{kind}_{dw}", bufs=3)
                    nc.vector.tensor_mul(t2[:, 0:WIN], pb[:], sin_t[:, slw])
                    nc.gpsimd.tensor_add(qk[(u, kind)][:, slw],
                                         t1[:, 0:WIN], t2[:, 0:WIN])
                return
            sl = slice(2 * wp * WIN, (2 * wp + 2) * WIN)
            pa = ps_exp.tile([128, 2 * WIN], F32, tag="ps_exp",
                             name=f"pa{b}_{wp}_{hl}_{kind}")
            nc.tensor.matmul(pa[:, 0:WIN], wqk_t[:, ja:ja + D], qt_w[i0][:])
            nc.tensor.matmul(pa[:, WIN:2 * WIN], wqk_t[:, ja:ja + D],
                             qt_w[i0 + 1][:])
            pb = ps_exp.tile([128, 2 * WIN], F32, tag="ps_exp",
                             name=f"pb{b}_{wp}_{hl}_{kind}")
            nc.tensor.matmul(pb[:, 0:WIN], wqk_t[:, ja + D:ja + 2 * D],
                             qt_w[i0][:])
            nc.tensor.matmul(pb[:, WIN:2 * WIN], wqk_t[:, ja + D:ja + 2 * D],
                             qt_w[i0 + 1][:])
            t1 = tmp.tile([128, 2 * WIN], F16, tag="t1",
                          name=f"t1_{b}_{wp}_{hl}_{kind}", bufs=3)
            nc.vector.tensor_mul(t1[:], pa[:], cos_t[:, sl])
            t2 = tmp.tile([128, 2 * WIN], F16, tag="t2",
                          name=f"t2_{b}_{wp}_{hl}_{kind}", bufs=3)
            nc.vector.tensor_mul(t2[:], pb[:], sin_t[:, sl])
            if b == 0 and wp == 0:
                # startup critical path: DVE add is ~4x faster than GpSimd
                nc.vector.tensor_add(qk[(u, kind)][:, sl], t1[:], t2[:])
            else:
                nc.gpsimd.tensor_add(qk[(u, kind)][:, sl], t1[:], t2[:])

        def psv_piece(b, w):
            i = b * NW + w
            for sub in range(4):
                psv = ps_misc.tile([128, WIN], F32, tag="misc",
                                   name=f"psv{b}_{w}_{sub}")
                nc.tensor.matmul(
                    psv[:, 0:HPC * D],
                    qt_w[i][:, sub * 128:(sub + 1) * 128], wv_t[:])
                nc.scalar.copy(vh[(b, 4 * w + sub)][:], psv[:, 0:HPC * D])

        def score_pair(b, W, hl, pair):
            u = b * HPC + hl
            par = u % 2
            ea = ea_t[(W, par)]
            qrow = qk[(u, 0)]
            krow = qk[(u, 1)]
            pt = ps_exp.tile([128, 2 * WIN], F32, tag="ps_exp",
                             name=f"pt{b}_{W}_{hl}_{pair}")
            jlos = []
            for k2 in range(2):
                tci = 2 * pair + k2
                jlo = max(0, tci * 128 - W * WIN)
                jlos.append(jlo)
                nc.tensor.matmul(
                    pt[:, k2 * WIN + jlo:(k2 + 1) * WIN],
                    krow[:, tci * 128:(tci + 1) * 128],
                    qrow[:, W * WIN + jlo:(W + 1) * WIN])
            if pair < 2 * W:
                nc.scalar.activation(
                    ea[:, 2 * pair * WIN:(2 * pair + 2) * WIN], pt[:],
                    AF.Exp, scale=float(SCALE))
            else:
                for k2 in range(2):
                    tci = 2 * pair + k2
                    jlo = jlos[k2]
                    nc.scalar.activation(
                        ea[:, tci * WIN + jlo:(tci + 1) * WIN],
                        pt[:, k2 * WIN + jlo:(k2 + 1) * WIN],
                        AF.Exp, scale=float(SCALE))
                    nc.gpsimd.tensor_mul(
                        ea[:, tci * WIN + jlo:tci * WIN + jlo + 128],
                        ea[:, tci * WIN + jlo:tci * WIN + jlo + 128],
                        tril_t[:])

        def fin_piece(b, W, hl):
            u = b * HPC + hl
            par = u % 2
            nch = 4 * W + 4
            ea = ea_t[(W, par)]
            scr = scr_t[(W, par)]
            # den tree (bf16 2x adds) -> scr[:, 0:WIN]
            n = nch
            nc.vector.tensor_add(scr[:, 0:(n // 2) * WIN],
                                 ea[:, 0:(n // 2) * WIN],
                                 ea[:, (n // 2) * WIN:n * WIN])
            n //= 2
            while n > 1:
                h = n // 2
                nc.vector.tensor_add(scr[:, 0:h * WIN], scr[:, 0:h * WIN],
                                     scr[:, h * WIN:2 * h * WIN])
                if n % 2:
                    nc.vector.tensor_add(scr[:, 0:WIN], scr[:, 0:WIN],
                                         scr[:, (n - 1) * WIN:n * WIN])
                n = h
            den_bc = ps_misc.tile([128, WIN], F32, tag="misc",
                                  name=f"den{b}_{W}_{hl}")
            nc.tensor.matmul(den_bc[:], ones_t[:], scr[:, 0:WIN])
            rden = tmp.tile([128, WIN], F32, tag="rden",
                            name=f"rden{b}_{W}_{hl}", bufs=3)
            nc.vector.reciprocal_approx_fast(out=rden[:], in_=den_bc[:])
            oT = ps_oT.tile([128, WIN], F32, tag="oT", name=f"oT{b}_{W}_{hl}")
            for tci in range(nch):
                jlo = max(0, tci * 128 - W * WIN)
                nc.tensor.matmul(
                    oT[:, jlo:WIN],
                    vh[(b, tci)][:, hl * D:(hl + 1) * D],
                    ea[:, tci * WIN + jlo:(tci + 1) * WIN],
                    start=(tci == 0), stop=(tci == nch - 1))
            oT_sb = tmp.tile([128, WIN], BF16, tag="oT_sb",
                             name=f"oTsb{b}_{W}_{hl}", bufs=2)
            nc.vector.tensor_mul(oT_sb[:], oT[:], rden[:])
            if hl == 0:
                fins[(b, W)] = ps_misc.tile([128, WIN], F32, tag="misc",
                                            name=f"fin{b}_{W}")
            fin = fins[(b, W)]
            nc.tensor.matmul(fin[:], wo_t[:, hl * E:(hl + 1) * E], oT_sb[:],
                             start=(hl == 0), stop=(hl == HPC - 1))
            if hl == HPC - 1:
                fin_sb = tmp.tile([128, WIN], F32, tag="fin_sb",
                                  name=f"fsb{b}_{W}", bufs=2)
                nc.scalar.copy(fin_sb[:], fin[:])
                nc.sync.dma_start(
                    outp_d[b * E:(b + 1) * E, W * WIN:(W + 1) * WIN],
                    fin_sb[:])

        def weave(c_pieces, b_pieces):
            """Interleave: one B piece after each of the first C pieces."""
            out = []
            bi = 0
            for i, cp in enumerate(c_pieces):
                out.append(cp)
                if bi < len(b_pieces):
                    out.append(b_pieces[bi])
                    bi += 1
            out.extend(b_pieces[bi:])
            return out

        def c_task(b, W, hl):
            nch = 4 * W + 4
            ps = [lambda b=b, W=W, hl=hl, p=p: score_pair(b, W, hl, p)
                  for p in range(nch // 2)]
            ps.append(lambda b=b, W=W, hl=hl: fin_piece(b, W, hl))
            return ps

        def b_task(b, wp, misc=False):
            ps = [lambda b=b, wp=wp, hl=hl, k=k, m=misc: rope_piece(b, wp, hl, k, m)
                  for hl in range(HPC) for k in range(2)]
            ps += [lambda b=b, w=w: psv_piece(b, w)
                   for w in (2 * wp, 2 * wp + 1)]
            return ps

        pieces = []
        pieces += b_task(0, 0)
        pieces += c_task(0, 1, 0) + c_task(0, 1, 1)
        pieces += c_task(0, 0, 0) + c_task(0, 0, 1)
        pieces += b_task(0, 1, misc=True)
        pieces += c_task(0, 2, 0) + c_task(0, 2, 1)
        pieces += b_task(1, 0, misc=True) + b_task(1, 1, misc=True)
        pieces += c_task(0, 3, 0) + c_task(0, 3, 1)
        pieces += c_task(1, 2, 0) + c_task(1, 2, 1)
        pieces += c_task(1, 3, 0) + c_task(1, 3, 1)
        pieces += c_task(1, 1, 0) + c_task(1, 1, 1)
        pieces += c_task(1, 0, 0) + c_task(1, 0, 1)
        for p in pieces:
            p()

    nc.compile()
    return nc


def _get_nc():
    if "nc" not in _CACHE:
        _CACHE["nc"] = _build()
    return _CACHE["nc"]


def _host_inputs(q, W_q, W_k, W_v, W_o):
    """Shared (core-independent) host-side prep."""
    qT = np.ascontiguousarray(q.reshape(B * S, E).T).astype(np.float16)

    half = D // 2
    inv = (1.0 / (10000.0 ** (np.arange(half, dtype=np.float64) * 2.0 / D)))
    ang = np.arange(S, dtype=np.float64)[None, :] * inv[:, None]   # [half, S]
    cosT = np.repeat(np.cos(ang), 2, axis=0).astype(np.float16)    # [D, S]
    sinT = np.repeat(np.sin(ang), 2, axis=0).astype(np.float16)
    tril = np.tril(np.ones((128, 128), dtype=np.float32)).T.astype(BF)
    tril = np.ascontiguousarray(tril)
    return qT, cosT, sinT, tril, None


def _swap_neg(w):
    """W' columns: w2[:, 2i] = -w[:, 2i+1], w2[:, 2i+1] = w[:, 2i]."""
    w2 = np.empty_like(w)
    w2[:, 0::2] = -w[:, 1::2]
    w2[:, 1::2] = w[:, 0::2]
    return w2


def kernel(q, W_q, W_k, W_v, W_o):
    q = np.asarray(q, dtype=np.float32)
    W_q = np.asarray(W_q, dtype=np.float32)
    W_k = np.asarray(W_k, dtype=np.float32)
    W_v = np.asarray(W_v, dtype=np.float32)
    W_o = np.asarray(W_o, dtype=np.float32)

    nc = _get_nc()
    qT, cosT, sinT, tril, _ = _host_inputs(q, W_q, W_k, W_v, W_o)

    in_maps = []
    for c in range(NCORES):
        wqk = np.empty((E, 8 * D), dtype=np.float16)
        wv = np.empty((E, HPC * D), dtype=np.float16)
        wo = np.empty((D, HPC * E), dtype=BF)
        for hl in range(HPC):
            h = c * HPC + hl
            for kind, Wm in ((0, W_q), (1, W_k)):
                wslc = Wm[:, h * D:(h + 1) * D]
                ja = (kind * 4 + hl * 2) * D
                wqk[:, ja:ja + D] = wslc.astype(np.float16)
                wqk[:, ja + D:ja + 2 * D] = _swap_neg(wslc).astype(np.float16)
            wv[:, hl * D:(hl + 1) * D] = W_v[:, h * D:(h + 1) * D]
            wo[:, hl * E:(hl + 1) * E] = W_o[h * D:(h + 1) * D, :].astype(BF)
        in_maps.append({
            "qT": qT, "wqk": wqk, "wv": wv, "wo": wo,
            "cosT": cosT, "sinT": sinT, "tril": tril,
        })

    res = bass_utils.run_bass_kernel_spmd(
        nc, in_maps, core_ids=list(range(NCORES)),
        trace=bool(int(os.environ.get("KERNEL_TRACE", "0"))))
    _CACHE["last_result"] = res

    acc = np.zeros((B * E, S), dtype=np.float64)
    for r in res.results:
        acc += r["outp"].astype(np.float64)
    out = acc.reshape(B, E, S).transpose(0, 2, 1).astype(np.float32)
    return out
